# revision 25
# baseline (speedup 1.0000x reference)
"""AttentionLayer Trainium2 kernel: 8-way SPMD (batch x query-half data parallel).

Per core (b = core//2, h = core%2), with x rotated so the core's query half
occupies columns 0..2047:
  k  = wk @ x + bk            [32, 4096]
  q  = wq @ x[:, :2048] + bq  [32, 2048]
  vT = x^T @ wv^T + bv        [4096, 256]   (v transposed, born in [j, c] layout)
  S^T[j, i] = k[:, j]^T q[:, i]   -> P = exp(S^T)  (softmax max-sub skipped:
                                     |S| <= ~15, safe in fp32)
  out[c, i] = (sum_j vT[j, c] P[j, i]) / (sum_j P[j, i]) + x[c, i]

Matmul chains run in bf16 (fast LDWEIGHTS + 1 cycle/row); PSUM accumulation
is fp32.  The K=32 score matmuls are 4x row-tiled (tile_position=(32r, 0)):
k and q live in 4 copies/strips across partition groups so 4 j-blocks of
scores compute concurrently.  q/k projections are 4x column-tiled
(tile_position=(0, 32r)) to produce those strip layouts directly.
The softmax denominator reduction runs as float32r.
"""
import numpy as np
import ml_dtypes

import concourse.bacc as bacc
import concourse.tile as tile
from concourse import mybir
from concourse.bass_utils import run_bass_kernel_spmd

F32 = mybir.dt.float32
F32R = mybir.dt.float32r
BF16 = mybir.dt.bfloat16
AF = mybir.ActivationFunctionType
ALU = mybir.AluOpType

C = 256          # channels
D = 32           # q/k dim (C // 8)
N = 4096         # h*w
NQ = 2048        # queries per core
NCORE = 8
NG = 8           # score groups per slice (4 j-blocks each)

_cache = {}


def _build():
    nc = bacc.Bacc(None, target_bir_lowering=False)
    xb_ext = nc.declare_dram_parameter("xb", [C, N], BF16, isOutput=False)
    xres_ext = nc.declare_dram_parameter("xres", [C, NQ], F32, isOutput=False)
    wqt_ext = nc.declare_dram_parameter("wqt", [C, D], BF16, isOutput=False)
    wkt_ext = nc.declare_dram_parameter("wkt", [C, D], BF16, isOutput=False)
    wvt_ext = nc.declare_dram_parameter("wvt", [C, C], BF16, isOutput=False)
    bq4_ext = nc.declare_dram_parameter("bq4", [128, 1], F32, isOutput=False)
    bk4_ext = nc.declare_dram_parameter("bk4", [128, 1], F32, isOutput=False)
    out_ext = nc.declare_dram_parameter("out", [C, NQ], F32, isOutput=True)

    with tile.TileContext(nc) as tc:
        with (
            tc.tile_pool(name="const", bufs=1) as const,
            tc.tile_pool(name="big", bufs=1) as big,
            tc.tile_pool(name="pbuf", bufs=6) as pbuf,
            tc.tile_pool(name="work", bufs=3) as work,
            tc.tile_pool(name="ps_sc", bufs=1, space="PSUM") as ps_sc,
            tc.tile_pool(name="ps_pv", bufs=1, space="PSUM") as ps_pv,
            tc.tile_pool(name="ps_small", bufs=1, space="PSUM") as ps_small,
            tc.tile_pool(name="ps_vt", bufs=1, space="PSUM") as ps_vt,
        ):
            wqt_sb = const.tile([128, 2 * D], BF16)
            wkt_sb = const.tile([128, 2 * D], BF16)
            wvt_sb = const.tile([128, 2 * C], BF16)
            bq4_sb = const.tile([128, 1], F32)
            bk4_sb = const.tile([128, 1], F32)
            ones_f = const.tile([128, 1], F32)
            ones_r = const.tile([128, 1], F32R)
            onesrow_f = const.tile([1, 128], F32)
            onesrow_r = const.tile([1, 128], F32R)

            x_sb = big.tile([128, 2 * N], BF16)       # ci blocks side by side
            xres_sb = big.tile([128, 2 * NQ], F32)
            # k4: strip r (partitions 32r..32r+31) holds j-blocks 4g+r at
            # free g*128..(g+1)*128
            k4_sb = big.tile([128, 1024], BF16)
            # q4: strip r holds a full copy of q (slices side by side)
            q4_sb = big.tile([128, NQ], BF16)
            vt_sb = big.tile([128, 32 * C], BF16)     # [j%128, jb*256 + c]

            # critical-path DMAs first: biases + k/q weights, then
            # first-half x (ci0 on sync queue, ci1 on scalar queue)
            nc.sync.dma_start(bk4_sb[:], bk4_ext[:])
            nc.sync.dma_start(bq4_sb[:], bq4_ext[:])
            for ci in range(2):
                nc.sync.dma_start(wkt_sb[:, ci * D:(ci + 1) * D],
                                  wkt_ext[ci * 128:(ci + 1) * 128, :])
                nc.scalar.dma_start(wqt_sb[:, ci * D:(ci + 1) * D],
                                    wqt_ext[ci * 128:(ci + 1) * 128, :])
            # all h0 x chunks on the sync queue so k/q/vt deps don't sit
            # behind the scalar queue's large transfers
            for s in range(4):
                for ci in range(2):
                    nc.sync.dma_start(
                        x_sb[:, ci * N + s * 512: ci * N + (s + 1) * 512],
                        xb_ext[ci * 128:(ci + 1) * 128, s * 512:(s + 1) * 512])
            for ci in range(2):
                nc.scalar.dma_start(wvt_sb[:, ci * C:(ci + 1) * C],
                                    wvt_ext[ci * 128:(ci + 1) * 128, :])
            nc.scalar.dma_start(x_sb[:, 2048:4096], xb_ext[0:128, 2048:4096])
            nc.scalar.dma_start(
                x_sb[:, N + 2048:2 * N], xb_ext[128:256, 2048:4096])
            nc.vector.memset(ones_f[:], 1.0)
            nc.vector.tensor_copy(ones_r[:], ones_f[:])
            nc.vector.memset(onesrow_f[:], 1.0)
            nc.vector.tensor_copy(onesrow_r[:], onesrow_f[:])
            for t in range(4):
                for ci in range(2):
                    nc.sync.dma_start(
                        xres_sb[:, ci * NQ + t * 512: ci * NQ + (t + 1) * 512],
                        xres_ext[ci * 128:(ci + 1) * 128, t * 512:(t + 1) * 512])

            def k_proj(gh):
                """Fill k4_sb[:, gh*512:(gh+1)*512] (j-blocks 16gh..16gh+15).

                Column-tiled: strip r gets blocks 4g+r, g in 4gh..4gh+3."""
                ps = ps_vt.tile([128, 512], F32, tag="vt", name="k_ps")
                for r in range(4):
                    for ci in range(2):
                        # rhs: x columns of blocks {4g+r : g in 4gh..4gh+3}
                        # block b at free offset b*128 = (4g+r)*128
                        base = ci * N + (16 * gh + r) * 128
                        rhs = x_sb[:, base: base + 13 * 128]
                        rhs = rhs.rearrange("p (g f) -> p g f", f=128)[:, 0:13:4, :]
                        nc.tensor.matmul(
                            ps[32 * r:32 * (r + 1), :],
                            wkt_sb[:, ci * D:(ci + 1) * D],
                            rhs,
                            start=(ci == 0), stop=(ci == 1),
                            tile_position=(0, 32 * r))
                nc.vector.tensor_scalar_add(
                    k4_sb[:, gh * 512:(gh + 1) * 512], ps[:], bk4_sb[:])

            def q_proj(t, on_vector=False):
                """Fill q4_sb[:, t*512:(t+1)*512]: q slice replicated in 4 strips."""
                ps = ps_small.tile([128, 512], F32, tag="small", name="q_ps")
                for r in range(4):
                    for ci in range(2):
                        nc.tensor.matmul(
                            ps[32 * r:32 * (r + 1), :],
                            wqt_sb[:, ci * D:(ci + 1) * D],
                            x_sb[:, ci * N + t * 512: ci * N + (t + 1) * 512],
                            start=(ci == 0), stop=(ci == 1),
                            tile_position=(0, 32 * r))
                if on_vector:
                    nc.vector.tensor_scalar_add(
                        q4_sb[:, t * 512:(t + 1) * 512], ps[:], bq4_sb[:])
                else:
                    nc.scalar.add(
                        q4_sb[:, t * 512:(t + 1) * 512], ps[:], bq4_sb[:])

            def vt_proj(jb):
                pool, tag = ((ps_vt, "vt") if jb % 2 == 0 else
                             (ps_small, "small"))
                vps = pool.tile([128, C], F32, tag=tag, name="vt_ps")
                for ci in range(2):
                    nc.tensor.matmul(
                        vps[:],
                        x_sb[:, ci * N + jb * 128: ci * N + (jb + 1) * 128],
                        wvt_sb[:, ci * C:(ci + 1) * C],
                        start=(ci == 0), stop=(ci == 1))
                nc.vector.tensor_copy(vt_sb[:, jb * C:(jb + 1) * C], vps[:])

            q_proj(0, on_vector=True)
            k_proj(0)

            pairs = [(t, g) for t in range(4) for g in range(NG)]
            accs = {}
            pvls = {}
            p_tiles = {}

            def scores_exp_r(t, g):
                if t == 0:
                    for r in range(4):
                        vt_proj(4 * g + r)
                    if g == 1:
                        k_proj(1)
                if g == 0:
                    pvls[t] = [
                        ps_pv.tile([128, 512], F32, tag=f"pv{cb}",
                                   name=f"pv{cb}")
                        for cb in range(2)]
                sc = ps_sc.tile([128, 2048], F32, tag="sc", name="sc")
                for r in range(4):
                    # j-block 4g+r lives in strip r at free g*128
                    nc.tensor.matmul(
                        sc[:, r * 512:(r + 1) * 512],
                        k4_sb[32 * r:32 * (r + 1), g * 128:(g + 1) * 128],
                        q4_sb[32 * r:32 * (r + 1), t * 512:(t + 1) * 512],
                        start=True, stop=True,
                        tile_position=(32 * r, 0))
                p_sb = pbuf.tile([128, 2048], BF16, tag="p", name="p_sb")
                nc.scalar.activation(p_sb[:], sc[:], AF.Exp)
                p_tiles[(t, g)] = p_sb
                # r-accumulation on DVE (bf16 pair-sum at 2x, then fp32)
                tmp = work.tile([128, 1024], BF16, tag="tmp", name="tmp")
                nc.vector.tensor_add(
                    tmp[:], p_sb[:, 0:1024], p_sb[:, 1024:2048])
                if g == 0:
                    acc = work.tile([128, 1024], F32, tag="acc", name="acc")
                    nc.vector.tensor_copy(acc[:], tmp[:])
                    accs[t] = acc
                else:
                    nc.vector.tensor_add(accs[t][:], accs[t][:], tmp[:])
                if (t, g) == (0, 3) or g == 3:
                    if t < 3:
                        q_proj(t + 1, on_vector=True)

            def pv_mm(t, g):
                p_sb = p_tiles.pop((t, g))
                pv = pvls[t]
                for r in range(4):
                    jb = 4 * g + r
                    for cb in range(2):
                        nc.tensor.matmul(
                            pv[cb][:],
                            vt_sb[:, jb * C + cb * 128: jb * C + (cb + 1) * 128],
                            p_sb[:, r * 512:(r + 1) * 512],
                            start=(g == 0 and r == 0),
                            stop=(g == NG - 1 and r == 3))

            epi = {}

            def epilogue_a(t):
                """After the last PV of slice t: fold r, free pv banks."""
                acc_r = work.tile([128, 512], F32R, tag="acc_r", name="acc_r")
                nc.vector.tensor_add(
                    acc_r[:], accs[t][:, 0:512], accs[t][:, 512:1024])
                rps = ps_vt.tile([1, 512], F32, tag="vt", name="rps")
                nc.tensor.matmul(rps[:], ones_r[:], acc_r[:],
                                 start=True, stop=True)
                rinv = work.tile([1, 512], F32, tag="rinv", name="rinv")
                nc.vector.reciprocal_approx_fast(rinv[:], rps[:])
                rinv_r = work.tile([1, 512], F32R, tag="rinv_r", name="rinv_r")
                nc.vector.tensor_copy(rinv_r[:], rinv[:])
                pvs = []
                for cb in range(2):
                    p_cp = work.tile([128, 512], F32, tag=f"pvs{cb}",
                                     name=f"pvs{cb}")
                    nc.vector.tensor_copy(p_cp[:], pvls[t][cb][:])
                    pvs.append(p_cp)
                epi[t] = (rinv_r, pvs)

            def epilogue_b(t):
                rinv_r, pvs = epi.pop(t)
                rbc = ps_small.tile([128, 512], F32, tag="small", name="rbc")
                nc.tensor.matmul(rbc[:], onesrow_r[:], rinv_r[:],
                                 start=True, stop=True)
                for cb in range(2):
                    o_tmp = work.tile([128, 512], F32, tag="o_tmp",
                                      name="o_tmp")
                    nc.vector.tensor_mul(o_tmp[:], pvs[cb][:], rbc[:])
                    o_out = work.tile([128, 512], F32, tag="o_out",
                                      name="o_out")
                    nc.vector.tensor_add(
                        o_out[:], o_tmp[:],
                        xres_sb[:, cb * NQ + t * 512: cb * NQ + (t + 1) * 512])
                    nc.sync.dma_start(
                        out_ext[cb * 128:(cb + 1) * 128,
                                t * 512:(t + 1) * 512],
                        o_out[:])

            for i in range(len(pairs) + 2):
                if i < len(pairs):
                    scores_exp_r(*pairs[i])
                if 1 <= i <= len(pairs):
                    tp, gp = pairs[i - 1]
                    pv_mm(tp, gp)
                    if gp == NG - 1:
                        epilogue_a(tp)
                if 2 <= i <= len(pairs) + 1:
                    tq, gq = pairs[i - 2]
                    if gq == NG - 1:
                        epilogue_b(tq)
    nc.compile()
    return nc


def _get_nc():
    if "nc" not in _cache:
        _cache["nc"] = _build()
    return _cache["nc"]


def _in_maps(x, wq, bq, wk, bk, wv, bv):
    wqt = np.ascontiguousarray(wq.T).astype(ml_dtypes.bfloat16)
    wkt = np.ascontiguousarray(wk.T).astype(ml_dtypes.bfloat16)
    wvt = np.ascontiguousarray(wv.T).astype(ml_dtypes.bfloat16)
    bq4 = np.ascontiguousarray(
        np.tile(np.asarray(bq, np.float32).reshape(D, 1), (4, 1)))
    bk4 = np.ascontiguousarray(
        np.tile(np.asarray(bk, np.float32).reshape(D, 1), (4, 1)))
    maps = []
    for core in range(NCORE):
        b, h = core // 2, core % 2
        xb = np.asarray(x[b], dtype=np.float32).reshape(C, N)
        if h == 1:
            xc = np.concatenate([xb[:, NQ:], xb[:, :NQ]], axis=1)
        else:
            xc = xb
        maps.append({
            "xb": np.ascontiguousarray(xc).astype(ml_dtypes.bfloat16),
            "xres": np.ascontiguousarray(
                xc[:, :NQ] + np.asarray(bv, np.float32).reshape(C, 1)),
            "wqt": wqt, "wkt": wkt, "wvt": wvt,
            "bq4": bq4, "bk4": bk4,
        })
    return maps


def _get_runner():
    """Build the SPMD graph once and cache a reusable jitted executable
    (run_bass_kernel_spmd re-jits per call, paying a full XLA compile)."""
    if "runner" in _cache:
        return _cache["runner"]
    import jax
    from jax.sharding import Mesh, PartitionSpec
    from jax.experimental.shard_map import shard_map
    from concourse import bass2jax, mybir as mb

    nc = _get_nc()
    bass2jax.install_neuronx_cc_hook()
    partition_name = (nc.partition_id_tensor.name
                      if nc.partition_id_tensor else None)
    in_names, out_names, out_avals, zero_shapes = [], [], [], []
    for alloc in nc.m.functions[0].allocations:
        if not isinstance(alloc, mb.MemoryLocationSet):
            continue
        name = alloc.memorylocations[0].name
        if alloc.kind == "ExternalInput":
            if name != partition_name:
                in_names.append(name)
        elif alloc.kind == "ExternalOutput":
            out_names.append(name)
            shape = tuple(alloc.tensor_shape)
            dtype = mb.dt.np(alloc.dtype)
            out_avals.append(jax.core.ShapedArray(shape, dtype))
            zero_shapes.append((shape, dtype))
    n_params = len(in_names)
    full_in_names = list(in_names) + list(out_names)
    if partition_name is not None:
        full_in_names.append(partition_name)
    donate = tuple(range(n_params, n_params + len(out_names)))

    def _body(*args):
        operands = list(args)
        if partition_name is not None:
            operands.append(bass2jax.partition_id_tensor())
        outs = bass2jax._bass_exec_p.bind(
            *operands,
            out_avals=tuple(out_avals),
            in_names=tuple(full_in_names),
            out_names=tuple(out_names),
            lowering_input_output_aliases=(),
            sim_require_finite=True,
            sim_require_nnan=True,
            nc=nc,
        )
        return tuple(outs)

    devices = jax.devices()[:NCORE]
    mesh = Mesh(np.asarray(devices), ("core",))
    in_specs = (PartitionSpec("core"),) * (n_params + len(out_names))
    out_specs = (PartitionSpec("core"),) * len(out_names)
    sharded = jax.jit(
        shard_map(_body, mesh=mesh, in_specs=in_specs, out_specs=out_specs,
                  check_rep=False),
        donate_argnums=donate, keep_unused=True)
    runner = (sharded, in_names, out_names, out_avals, zero_shapes)
    _cache["runner"] = runner
    return runner


def _run_fast(maps):
    sharded, in_names, out_names, out_avals, zero_shapes = _get_runner()
    concat_in = [
        np.concatenate([np.asarray(maps[c][name]) for c in range(NCORE)], axis=0)
        for name in in_names
    ]
    concat_zeros = [
        np.zeros((NCORE * s[0], *s[1:]), dt) for s, dt in zero_shapes
    ]
    out_arrs = sharded(*concat_in, *concat_zeros)
    return [
        {name: np.asarray(out_arrs[i]).reshape(NCORE, *out_avals[i].shape)[c]
         for i, name in enumerate(out_names)}
        for c in range(NCORE)
    ]


def _assemble(results):
    out = np.empty((4, C, N), dtype=np.float32)
    for core in range(NCORE):
        b, h = core // 2, core % 2
        out[b][:, h * NQ:(h + 1) * NQ] = results[core]["out"]
    return out.reshape(4, C, 64, 64)


def _run(inputs, trace=False, tmpdir=None):
    maps = _in_maps(**inputs)
    if trace:
        nc = _get_nc()
        res = run_bass_kernel_spmd(nc, maps, core_ids=list(range(NCORE)),
                                   trace=trace, tmpdir=tmpdir)
        return _assemble(res.results), res
    return _assemble(_run_fast(maps)), None


def kernel(**inputs):
    out, _ = _run(inputs)
    return out


# revision 26
# speedup vs baseline: 1.1248x; 1.1248x over previous
"""AttentionLayer Trainium2 kernel: 8-way SPMD (batch x query-half data parallel).

Per core (b = core//2, h = core%2), with x rotated so the core's query half
occupies columns 0..2047:
  k  = wk @ x + bk            [32, 4096]
  q  = wq @ x[:, :2048] + bq  [32, 2048]
  vT = x^T @ wv^T + bv        [4096, 256]   (v transposed, born in [j, c] layout)
  S^T[j, i] = k[:, j]^T q[:, i]   -> P = exp(S^T)  (softmax max-sub skipped:
                                     |S| <= ~15, safe in fp32)
  out[c, i] = (sum_j vT[j, c] P[j, i]) / (sum_j P[j, i]) + x[c, i]

Matmul chains run in bf16 (fast LDWEIGHTS + 1 cycle/row); PSUM accumulation
is fp32.  The K=32 score matmuls are 4x row-tiled (tile_position=(32r, 0)):
k and q live in 4 copies/strips across partition groups so 4 j-blocks of
scores compute concurrently.  q/k projections are 4x column-tiled
(tile_position=(0, 32r)) to produce those strip layouts directly.
The softmax denominator reduction runs as float32r.
"""
import numpy as np
import ml_dtypes

import concourse.bacc as bacc
import concourse.tile as tile
from concourse import mybir
from concourse.bass_utils import run_bass_kernel_spmd

F32 = mybir.dt.float32
F32R = mybir.dt.float32r
BF16 = mybir.dt.bfloat16
AF = mybir.ActivationFunctionType
ALU = mybir.AluOpType

C = 256          # channels
D = 32           # q/k dim (C // 8)
N = 4096         # h*w
NQ = 2048        # queries per core
NCORE = 8
NG = 8           # score groups per slice (4 j-blocks each)

_cache = {}


def _build():
    nc = bacc.Bacc(None, target_bir_lowering=False)
    xb_ext = nc.declare_dram_parameter("xb", [C, N], BF16, isOutput=False)
    xres_ext = nc.declare_dram_parameter("xres", [C, NQ], F32, isOutput=False)
    wqt_ext = nc.declare_dram_parameter("wqt", [C, D], BF16, isOutput=False)
    wkt_ext = nc.declare_dram_parameter("wkt", [C, D], BF16, isOutput=False)
    wvt_ext = nc.declare_dram_parameter("wvt", [C, C], BF16, isOutput=False)
    bq4_ext = nc.declare_dram_parameter("bq4", [128, 1], F32, isOutput=False)
    bk4_ext = nc.declare_dram_parameter("bk4", [128, 1], F32, isOutput=False)
    out_ext = nc.declare_dram_parameter("out", [C, NQ], F32, isOutput=True)

    with tile.TileContext(nc) as tc:
        with (
            tc.tile_pool(name="const", bufs=1) as const,
            tc.tile_pool(name="big", bufs=1) as big,
            tc.tile_pool(name="pbuf", bufs=6) as pbuf,
            tc.tile_pool(name="work", bufs=3) as work,
            tc.tile_pool(name="ps_sc", bufs=1, space="PSUM") as ps_sc,
            tc.tile_pool(name="ps_pv", bufs=1, space="PSUM") as ps_pv,
            tc.tile_pool(name="ps_small", bufs=1, space="PSUM") as ps_small,
            tc.tile_pool(name="ps_vt", bufs=1, space="PSUM") as ps_vt,
        ):
            wqt_sb = const.tile([128, 2 * D], BF16)
            wkt_sb = const.tile([128, 2 * D], BF16)
            wvt_sb = const.tile([128, 2 * C], BF16)
            bq4_sb = const.tile([128, 1], F32)
            bk4_sb = const.tile([128, 1], F32)
            ones_f = const.tile([128, 1], F32)
            ones_r = const.tile([128, 1], F32R)
            onesrow_f = const.tile([1, 128], F32)
            onesrow_r = const.tile([1, 128], F32R)

            x_sb = big.tile([128, 2 * N], BF16)       # ci blocks side by side
            xres_sb = big.tile([128, 2 * NQ], F32)
            # k4: strip r (partitions 32r..32r+31) holds j-blocks 4g+r at
            # free g*128..(g+1)*128
            k4_sb = big.tile([128, 1024], BF16)
            # q4: strip r holds a full copy of q (slices side by side)
            q4_sb = big.tile([128, NQ], BF16)
            vt_sb = big.tile([128, 32 * C], BF16)     # [j%128, jb*256 + c]

            # critical-path DMAs first: biases + k/q weights, then
            # first-half x (ci0 on sync queue, ci1 on scalar queue)
            nc.sync.dma_start(bk4_sb[:], bk4_ext[:])
            nc.sync.dma_start(bq4_sb[:], bq4_ext[:])
            for ci in range(2):
                nc.sync.dma_start(wkt_sb[:, ci * D:(ci + 1) * D],
                                  wkt_ext[ci * 128:(ci + 1) * 128, :])
                nc.scalar.dma_start(wqt_sb[:, ci * D:(ci + 1) * D],
                                    wqt_ext[ci * 128:(ci + 1) * 128, :])
            # all h0 x chunks on the sync queue so k/q/vt deps don't sit
            # behind the scalar queue's large transfers
            for s in range(4):
                for ci in range(2):
                    nc.sync.dma_start(
                        x_sb[:, ci * N + s * 512: ci * N + (s + 1) * 512],
                        xb_ext[ci * 128:(ci + 1) * 128, s * 512:(s + 1) * 512])
            for ci in range(2):
                nc.scalar.dma_start(wvt_sb[:, ci * C:(ci + 1) * C],
                                    wvt_ext[ci * 128:(ci + 1) * 128, :])
            nc.scalar.dma_start(x_sb[:, 2048:4096], xb_ext[0:128, 2048:4096])
            nc.scalar.dma_start(
                x_sb[:, N + 2048:2 * N], xb_ext[128:256, 2048:4096])
            nc.vector.memset(ones_f[:], 1.0)
            nc.vector.tensor_copy(ones_r[:], ones_f[:])
            nc.vector.memset(onesrow_f[:], 1.0)
            nc.vector.tensor_copy(onesrow_r[:], onesrow_f[:])
            for t in range(4):
                for ci in range(2):
                    nc.sync.dma_start(
                        xres_sb[:, ci * NQ + t * 512: ci * NQ + (t + 1) * 512],
                        xres_ext[ci * 128:(ci + 1) * 128, t * 512:(t + 1) * 512])

            def k_proj(gh):
                """Fill k4_sb[:, gh*512:(gh+1)*512] (j-blocks 16gh..16gh+15).

                Column-tiled: strip r gets blocks 4g+r, g in 4gh..4gh+3."""
                ps = ps_vt.tile([128, 512], F32, tag="vt", name="k_ps")
                for r in range(4):
                    for ci in range(2):
                        # rhs: x columns of blocks {4g+r : g in 4gh..4gh+3}
                        # block b at free offset b*128 = (4g+r)*128
                        base = ci * N + (16 * gh + r) * 128
                        rhs = x_sb[:, base: base + 13 * 128]
                        rhs = rhs.rearrange("p (g f) -> p g f", f=128)[:, 0:13:4, :]
                        nc.tensor.matmul(
                            ps[32 * r:32 * (r + 1), :],
                            wkt_sb[:, ci * D:(ci + 1) * D],
                            rhs,
                            start=(ci == 0), stop=(ci == 1),
                            tile_position=(0, 32 * r))
                nc.vector.tensor_scalar_add(
                    k4_sb[:, gh * 512:(gh + 1) * 512], ps[:], bk4_sb[:])

            def q_proj(t, on_vector=False):
                """Fill q4_sb[:, t*512:(t+1)*512]: q slice replicated in 4 strips."""
                ps = ps_small.tile([128, 512], F32, tag="small", name="q_ps")
                for r in range(4):
                    for ci in range(2):
                        nc.tensor.matmul(
                            ps[32 * r:32 * (r + 1), :],
                            wqt_sb[:, ci * D:(ci + 1) * D],
                            x_sb[:, ci * N + t * 512: ci * N + (t + 1) * 512],
                            start=(ci == 0), stop=(ci == 1),
                            tile_position=(0, 32 * r))
                if on_vector:
                    nc.vector.tensor_scalar_add(
                        q4_sb[:, t * 512:(t + 1) * 512], ps[:], bq4_sb[:])
                else:
                    nc.scalar.add(
                        q4_sb[:, t * 512:(t + 1) * 512], ps[:], bq4_sb[:])

            def vt_proj(jb):
                vps = ps_vt.tile([128, C], F32, tag="vt", name="vt_ps")
                for ci in range(2):
                    nc.tensor.matmul(
                        vps[:],
                        x_sb[:, ci * N + jb * 128: ci * N + (jb + 1) * 128],
                        wvt_sb[:, ci * C:(ci + 1) * C],
                        start=(ci == 0), stop=(ci == 1))
                nc.vector.tensor_copy(vt_sb[:, jb * C:(jb + 1) * C], vps[:])

            q_proj(0, on_vector=True)
            k_proj(0)

            pairs = [(t, g) for t in range(4) for g in range(NG)]
            accs = {}
            pvls = {}
            p_tiles = {}

            def scores_exp_r(t, g):
                if t == 0:
                    for r in range(4):
                        vt_proj(4 * g + r)
                    if g == 1:
                        k_proj(1)
                if g == 0:
                    pvls[t] = [
                        ps_pv.tile([128, 512], F32, tag=f"pv{cb}",
                                   name=f"pv{cb}")
                        for cb in range(2)]
                sc = ps_sc.tile([128, 2048], F32, tag="sc", name="sc")
                for r in range(4):
                    # j-block 4g+r lives in strip r at free g*128
                    nc.tensor.matmul(
                        sc[:, r * 512:(r + 1) * 512],
                        k4_sb[32 * r:32 * (r + 1), g * 128:(g + 1) * 128],
                        q4_sb[32 * r:32 * (r + 1), t * 512:(t + 1) * 512],
                        start=True, stop=True,
                        tile_position=(32 * r, 0))
                p_sb = pbuf.tile([128, 2048], BF16, tag="p", name="p_sb")
                nc.scalar.activation(p_sb[:], sc[:], AF.Exp)
                p_tiles[(t, g)] = p_sb
                # r-accumulation on DVE (bf16 pair-sum at 2x, then fp32)
                tmp = work.tile([128, 1024], BF16, tag="tmp", name="tmp")
                nc.vector.tensor_add(
                    tmp[:], p_sb[:, 0:1024], p_sb[:, 1024:2048])
                if g == 0:
                    acc = work.tile([128, 1024], F32, tag="acc", name="acc")
                    nc.vector.tensor_copy(acc[:], tmp[:])
                    accs[t] = acc
                else:
                    nc.vector.tensor_add(accs[t][:], accs[t][:], tmp[:])
                if (t, g) == (0, 3) or g == 3:
                    if t < 3:
                        q_proj(t + 1, on_vector=True)

            def pv_mm(t, g):
                p_sb = p_tiles.pop((t, g))
                pv = pvls[t]
                for r in range(4):
                    jb = 4 * g + r
                    for cb in range(2):
                        nc.tensor.matmul(
                            pv[cb][:],
                            vt_sb[:, jb * C + cb * 128: jb * C + (cb + 1) * 128],
                            p_sb[:, r * 512:(r + 1) * 512],
                            start=(g == 0 and r == 0),
                            stop=(g == NG - 1 and r == 3))

            epi = {}

            def epilogue_a(t):
                """After the last PV of slice t: fold r, free pv banks."""
                acc_r = work.tile([128, 512], F32R, tag="acc_r", name="acc_r")
                nc.vector.tensor_add(
                    acc_r[:], accs[t][:, 0:512], accs[t][:, 512:1024])
                rps = ps_vt.tile([1, 512], F32, tag="vt", name="rps")
                nc.tensor.matmul(rps[:], ones_r[:], acc_r[:],
                                 start=True, stop=True)
                rinv = work.tile([1, 512], F32, tag="rinv", name="rinv")
                nc.vector.reciprocal_approx_fast(rinv[:], rps[:])
                rinv_r = work.tile([1, 512], F32R, tag="rinv_r", name="rinv_r")
                nc.vector.tensor_copy(rinv_r[:], rinv[:])
                pvs = []
                for cb in range(2):
                    p_cp = work.tile([128, 512], F32, tag=f"pvs{cb}",
                                     name=f"pvs{cb}")
                    nc.vector.tensor_copy(p_cp[:], pvls[t][cb][:])
                    pvs.append(p_cp)
                epi[t] = (rinv_r, pvs)

            def epilogue_b(t):
                rinv_r, pvs = epi.pop(t)
                rbc = ps_small.tile([128, 512], F32, tag="small", name="rbc")
                nc.tensor.matmul(rbc[:], onesrow_r[:], rinv_r[:],
                                 start=True, stop=True)
                for cb in range(2):
                    o_tmp = work.tile([128, 512], F32, tag="o_tmp",
                                      name="o_tmp")
                    nc.vector.tensor_mul(o_tmp[:], pvs[cb][:], rbc[:])
                    o_out = work.tile([128, 512], F32, tag="o_out",
                                      name="o_out")
                    nc.vector.tensor_add(
                        o_out[:], o_tmp[:],
                        xres_sb[:, cb * NQ + t * 512: cb * NQ + (t + 1) * 512])
                    nc.sync.dma_start(
                        out_ext[cb * 128:(cb + 1) * 128,
                                t * 512:(t + 1) * 512],
                        o_out[:])

            for i in range(len(pairs) + 2):
                if i < len(pairs):
                    scores_exp_r(*pairs[i])
                if 1 <= i <= len(pairs):
                    tp, gp = pairs[i - 1]
                    pv_mm(tp, gp)
                    if gp == NG - 1:
                        epilogue_a(tp)
                if 2 <= i <= len(pairs) + 1:
                    tq, gq = pairs[i - 2]
                    if gq == NG - 1:
                        epilogue_b(tq)
    nc.compile()
    return nc


def _get_nc():
    if "nc" not in _cache:
        _cache["nc"] = _build()
    return _cache["nc"]


def _in_maps(x, wq, bq, wk, bk, wv, bv):
    wqt = np.ascontiguousarray(wq.T).astype(ml_dtypes.bfloat16)
    wkt = np.ascontiguousarray(wk.T).astype(ml_dtypes.bfloat16)
    wvt = np.ascontiguousarray(wv.T).astype(ml_dtypes.bfloat16)
    bq4 = np.ascontiguousarray(
        np.tile(np.asarray(bq, np.float32).reshape(D, 1), (4, 1)))
    bk4 = np.ascontiguousarray(
        np.tile(np.asarray(bk, np.float32).reshape(D, 1), (4, 1)))
    maps = []
    for core in range(NCORE):
        b, h = core // 2, core % 2
        xb = np.asarray(x[b], dtype=np.float32).reshape(C, N)
        if h == 1:
            xc = np.concatenate([xb[:, NQ:], xb[:, :NQ]], axis=1)
        else:
            xc = xb
        maps.append({
            "xb": np.ascontiguousarray(xc).astype(ml_dtypes.bfloat16),
            "xres": np.ascontiguousarray(
                xc[:, :NQ] + np.asarray(bv, np.float32).reshape(C, 1)),
            "wqt": wqt, "wkt": wkt, "wvt": wvt,
            "bq4": bq4, "bk4": bk4,
        })
    return maps


def _get_runner():
    """Build the SPMD graph once and cache a reusable jitted executable
    (run_bass_kernel_spmd re-jits per call, paying a full XLA compile)."""
    if "runner" in _cache:
        return _cache["runner"]
    import jax
    from jax.sharding import Mesh, PartitionSpec
    from jax.experimental.shard_map import shard_map
    from concourse import bass2jax, mybir as mb

    nc = _get_nc()
    bass2jax.install_neuronx_cc_hook()
    partition_name = (nc.partition_id_tensor.name
                      if nc.partition_id_tensor else None)
    in_names, out_names, out_avals, zero_shapes = [], [], [], []
    for alloc in nc.m.functions[0].allocations:
        if not isinstance(alloc, mb.MemoryLocationSet):
            continue
        name = alloc.memorylocations[0].name
        if alloc.kind == "ExternalInput":
            if name != partition_name:
                in_names.append(name)
        elif alloc.kind == "ExternalOutput":
            out_names.append(name)
            shape = tuple(alloc.tensor_shape)
            dtype = mb.dt.np(alloc.dtype)
            out_avals.append(jax.core.ShapedArray(shape, dtype))
            zero_shapes.append((shape, dtype))
    n_params = len(in_names)
    full_in_names = list(in_names) + list(out_names)
    if partition_name is not None:
        full_in_names.append(partition_name)
    donate = tuple(range(n_params, n_params + len(out_names)))

    def _body(*args):
        operands = list(args)
        if partition_name is not None:
            operands.append(bass2jax.partition_id_tensor())
        outs = bass2jax._bass_exec_p.bind(
            *operands,
            out_avals=tuple(out_avals),
            in_names=tuple(full_in_names),
            out_names=tuple(out_names),
            lowering_input_output_aliases=(),
            sim_require_finite=True,
            sim_require_nnan=True,
            nc=nc,
        )
        return tuple(outs)

    devices = jax.devices()[:NCORE]
    mesh = Mesh(np.asarray(devices), ("core",))
    in_specs = (PartitionSpec("core"),) * (n_params + len(out_names))
    out_specs = (PartitionSpec("core"),) * len(out_names)
    sharded = jax.jit(
        shard_map(_body, mesh=mesh, in_specs=in_specs, out_specs=out_specs,
                  check_rep=False),
        donate_argnums=donate, keep_unused=True)
    runner = (sharded, in_names, out_names, out_avals, zero_shapes)
    _cache["runner"] = runner
    return runner


def _run_fast(maps):
    sharded, in_names, out_names, out_avals, zero_shapes = _get_runner()
    concat_in = [
        np.concatenate([np.asarray(maps[c][name]) for c in range(NCORE)], axis=0)
        for name in in_names
    ]
    concat_zeros = [
        np.zeros((NCORE * s[0], *s[1:]), dt) for s, dt in zero_shapes
    ]
    out_arrs = sharded(*concat_in, *concat_zeros)
    return [
        {name: np.asarray(out_arrs[i]).reshape(NCORE, *out_avals[i].shape)[c]
         for i, name in enumerate(out_names)}
        for c in range(NCORE)
    ]


def _assemble(results):
    out = np.empty((4, C, N), dtype=np.float32)
    for core in range(NCORE):
        b, h = core // 2, core % 2
        out[b][:, h * NQ:(h + 1) * NQ] = results[core]["out"]
    return out.reshape(4, C, 64, 64)


def _run(inputs, trace=False, tmpdir=None):
    maps = _in_maps(**inputs)
    if trace:
        nc = _get_nc()
        res = run_bass_kernel_spmd(nc, maps, core_ids=list(range(NCORE)),
                                   trace=trace, tmpdir=tmpdir)
        return _assemble(res.results), res
    return _assemble(_run_fast(maps)), None


def kernel(**inputs):
    out, _ = _run(inputs)
    return out


# revision 27
# speedup vs baseline: 1.1445x; 1.0175x over previous
"""AttentionLayer Trainium2 kernel: 8-way SPMD (batch x query-half data parallel).

Per core (b = core//2, h = core%2), with x rotated so the core's query half
occupies columns 0..2047:
  k  = wk @ x + bk            [32, 4096]
  q  = wq @ x[:, :2048] + bq  [32, 2048]
  vT = x^T @ wv^T + bv        [4096, 256]   (v transposed, born in [j, c] layout)
  S^T[j, i] = k[:, j]^T q[:, i]   -> P = exp(S^T)  (softmax max-sub skipped:
                                     |S| <= ~15, safe in fp32)
  out[c, i] = (sum_j vT[j, c] P[j, i]) / (sum_j P[j, i]) + x[c, i]

Matmul chains run in bf16 (fast LDWEIGHTS + 1 cycle/row); PSUM accumulation
is fp32.  The K=32 score matmuls are 4x row-tiled (tile_position=(32r, 0)):
k and q live in 4 copies/strips across partition groups so 4 j-blocks of
scores compute concurrently.  q/k projections are 4x column-tiled
(tile_position=(0, 32r)) to produce those strip layouts directly.
The softmax denominator reduction runs as float32r.
"""
import numpy as np
import ml_dtypes

import concourse.bacc as bacc
import concourse.tile as tile
from concourse import mybir
from concourse.bass_utils import run_bass_kernel_spmd

F32 = mybir.dt.float32
F32R = mybir.dt.float32r
BF16 = mybir.dt.bfloat16
AF = mybir.ActivationFunctionType
ALU = mybir.AluOpType

C = 256          # channels
D = 32           # q/k dim (C // 8)
N = 4096         # h*w
NQ = 2048        # queries per core
NCORE = 8
NG = 8           # score groups per slice (4 j-blocks each)

_cache = {}


def _build():
    nc = bacc.Bacc(None, target_bir_lowering=False)
    xb_ext = nc.declare_dram_parameter("xb", [C, N], BF16, isOutput=False)
    xres_ext = nc.declare_dram_parameter("xres", [C, NQ], F32, isOutput=False)
    wqt_ext = nc.declare_dram_parameter("wqt", [C, D], BF16, isOutput=False)
    wkt_ext = nc.declare_dram_parameter("wkt", [C, D], BF16, isOutput=False)
    wvt_ext = nc.declare_dram_parameter("wvt", [C, C], BF16, isOutput=False)
    bq4_ext = nc.declare_dram_parameter("bq4", [128, 1], F32, isOutput=False)
    bk4_ext = nc.declare_dram_parameter("bk4", [128, 1], F32, isOutput=False)
    out_ext = nc.declare_dram_parameter("out", [C, NQ], F32, isOutput=True)

    with tile.TileContext(nc) as tc:
        with (
            tc.tile_pool(name="const", bufs=1) as const,
            tc.tile_pool(name="big", bufs=1) as big,
            tc.tile_pool(name="pbuf", bufs=6) as pbuf,
            tc.tile_pool(name="work", bufs=3) as work,
            tc.tile_pool(name="ps_sc", bufs=1, space="PSUM") as ps_sc,
            tc.tile_pool(name="ps_pv", bufs=1, space="PSUM") as ps_pv,
            tc.tile_pool(name="ps_small", bufs=1, space="PSUM") as ps_small,
            tc.tile_pool(name="ps_vt", bufs=1, space="PSUM") as ps_vt,
        ):
            wqt_sb = const.tile([128, 2 * D], BF16)
            wkt_sb = const.tile([128, 2 * D], BF16)
            wvt_sb = const.tile([128, 2 * C], BF16)
            bq4_sb = const.tile([128, 1], F32)
            bk4_sb = const.tile([128, 1], F32)
            ones_f = const.tile([128, 1], F32)
            ones_r = const.tile([128, 1], F32R)
            onesrow_f = const.tile([1, 128], F32)
            onesrow_r = const.tile([1, 128], F32R)

            x_sb = big.tile([128, 2 * N], BF16)       # ci blocks side by side
            xres_sb = big.tile([128, 2 * NQ], F32)
            # k4: strip r (partitions 32r..32r+31) holds j-blocks 4g+r at
            # free g*128..(g+1)*128
            k4_sb = big.tile([128, 1024], BF16)
            # q4: strip r holds a full copy of q (slices side by side)
            q4_sb = big.tile([128, NQ], BF16)
            vt_sb = big.tile([128, 32 * C], BF16)     # [j%128, jb*256 + c]

            # critical-path DMAs first: biases + k/q weights, then
            # first-half x (ci0 on sync queue, ci1 on scalar queue)
            nc.sync.dma_start(bk4_sb[:], bk4_ext[:])
            nc.sync.dma_start(bq4_sb[:], bq4_ext[:])
            for ci in range(2):
                nc.sync.dma_start(wkt_sb[:, ci * D:(ci + 1) * D],
                                  wkt_ext[ci * 128:(ci + 1) * 128, :])
                nc.scalar.dma_start(wqt_sb[:, ci * D:(ci + 1) * D],
                                    wqt_ext[ci * 128:(ci + 1) * 128, :])
            # all h0 x chunks on the sync queue so k/q/vt deps don't sit
            # behind the scalar queue's large transfers
            for s in range(4):
                for ci in range(2):
                    nc.sync.dma_start(
                        x_sb[:, ci * N + s * 512: ci * N + (s + 1) * 512],
                        xb_ext[ci * 128:(ci + 1) * 128, s * 512:(s + 1) * 512])
            for ci in range(2):
                nc.scalar.dma_start(wvt_sb[:, ci * C:(ci + 1) * C],
                                    wvt_ext[ci * 128:(ci + 1) * 128, :])
            nc.scalar.dma_start(x_sb[:, 2048:4096], xb_ext[0:128, 2048:4096])
            nc.scalar.dma_start(
                x_sb[:, N + 2048:2 * N], xb_ext[128:256, 2048:4096])
            nc.vector.memset(ones_f[:], 1.0)
            nc.vector.tensor_copy(ones_r[:], ones_f[:])
            nc.vector.memset(onesrow_f[:], 1.0)
            nc.vector.tensor_copy(onesrow_r[:], onesrow_f[:])
            for t in range(4):
                for ci in range(2):
                    nc.sync.dma_start(
                        xres_sb[:, ci * NQ + t * 512: ci * NQ + (t + 1) * 512],
                        xres_ext[ci * 128:(ci + 1) * 128, t * 512:(t + 1) * 512])

            def k_proj(gh):
                """Fill k4_sb[:, gh*512:(gh+1)*512] (j-blocks 16gh..16gh+15).

                Column-tiled: strip r gets blocks 4g+r, g in 4gh..4gh+3."""
                ps = ps_vt.tile([128, 512], F32, tag="vt", name="k_ps")
                for r in range(4):
                    for ci in range(2):
                        # rhs: x columns of blocks {4g+r : g in 4gh..4gh+3}
                        # block b at free offset b*128 = (4g+r)*128
                        base = ci * N + (16 * gh + r) * 128
                        rhs = x_sb[:, base: base + 13 * 128]
                        rhs = rhs.rearrange("p (g f) -> p g f", f=128)[:, 0:13:4, :]
                        nc.tensor.matmul(
                            ps[32 * r:32 * (r + 1), :],
                            wkt_sb[:, ci * D:(ci + 1) * D],
                            rhs,
                            start=(ci == 0), stop=(ci == 1),
                            tile_position=(0, 32 * r))
                nc.vector.tensor_scalar_add(
                    k4_sb[:, gh * 512:(gh + 1) * 512], ps[:], bk4_sb[:])

            def q_proj(t, on_vector=False):
                """Fill q4_sb[:, t*512:(t+1)*512]: q slice replicated in 4 strips."""
                ps = ps_small.tile([128, 512], F32, tag="small", name="q_ps")
                for r in range(4):
                    for ci in range(2):
                        nc.tensor.matmul(
                            ps[32 * r:32 * (r + 1), :],
                            wqt_sb[:, ci * D:(ci + 1) * D],
                            x_sb[:, ci * N + t * 512: ci * N + (t + 1) * 512],
                            start=(ci == 0), stop=(ci == 1),
                            tile_position=(0, 32 * r))
                if on_vector:
                    nc.vector.tensor_scalar_add(
                        q4_sb[:, t * 512:(t + 1) * 512], ps[:], bq4_sb[:])
                else:
                    nc.scalar.add(
                        q4_sb[:, t * 512:(t + 1) * 512], ps[:], bq4_sb[:])

            def vt_proj(jb):
                vps = ps_vt.tile([128, C], F32, tag="vt", name="vt_ps")
                for ci in range(2):
                    nc.tensor.matmul(
                        vps[:],
                        x_sb[:, ci * N + jb * 128: ci * N + (jb + 1) * 128],
                        wvt_sb[:, ci * C:(ci + 1) * C],
                        start=(ci == 0), stop=(ci == 1))
                nc.vector.tensor_copy(vt_sb[:, jb * C:(jb + 1) * C], vps[:])

            q_proj(0, on_vector=True)
            k_proj(0)

            pairs = [(t, g) for t in range(4) for g in range(NG)]
            accs = {}
            pvls = {}
            p_tiles = {}

            def emit_r_tts(t, g):
                p_sb = p_tiles[(t, g)]
                tmp = work.tile([128, 1024], BF16, tag="tmp", name="tmp")
                nc.vector.tensor_add(
                    tmp[:], p_sb[:, 0:1024], p_sb[:, 1024:2048])
                if g == 0:
                    acc = work.tile([128, 1024], F32, tag="acc", name="acc")
                    nc.vector.tensor_copy(acc[:], tmp[:])
                    accs[t] = acc
                else:
                    nc.vector.tensor_add(accs[t][:], accs[t][:], tmp[:])

            def scores_exp_r(t, g):
                if t == 0:
                    for r in range(4):
                        vt_proj(4 * g + r)
                    if g == 1:
                        k_proj(1)
                if g == 0:
                    pvls[t] = [
                        ps_pv.tile([128, 512], F32, tag=f"pv{cb}",
                                   name=f"pv{cb}")
                        for cb in range(2)]
                sc = ps_sc.tile([128, 2048], F32, tag="sc", name="sc")
                for r in range(4):
                    # j-block 4g+r lives in strip r at free g*128
                    nc.tensor.matmul(
                        sc[:, r * 512:(r + 1) * 512],
                        k4_sb[32 * r:32 * (r + 1), g * 128:(g + 1) * 128],
                        q4_sb[32 * r:32 * (r + 1), t * 512:(t + 1) * 512],
                        start=True, stop=True,
                        tile_position=(32 * r, 0))
                p_sb = pbuf.tile([128, 2048], BF16, tag="p", name="p_sb")
                nc.scalar.activation(p_sb[:], sc[:], AF.Exp)
                p_tiles[(t, g)] = p_sb
                if not (g == 0 and t > 0):
                    emit_r_tts(t, g)
                if (t, g) == (0, 3) or g == 3:
                    if t < 3:
                        q_proj(t + 1, on_vector=True)

            def pv_mm(t, g):
                p_sb = p_tiles[(t, g)]
                pv = pvls[t]
                for r in range(4):
                    jb = 4 * g + r
                    for cb in range(2):
                        nc.tensor.matmul(
                            pv[cb][:],
                            vt_sb[:, jb * C + cb * 128: jb * C + (cb + 1) * 128],
                            p_sb[:, r * 512:(r + 1) * 512],
                            start=(g == 0 and r == 0),
                            stop=(g == NG - 1 and r == 3))

            epi = {}

            def epilogue_a(t):
                """After the last PV of slice t: fold r, free pv banks."""
                acc_r = work.tile([128, 512], F32R, tag="acc_r", name="acc_r")
                nc.vector.tensor_add(
                    acc_r[:], accs[t][:, 0:512], accs[t][:, 512:1024])
                rps = ps_vt.tile([1, 512], F32, tag="vt", name="rps")
                nc.tensor.matmul(rps[:], ones_r[:], acc_r[:],
                                 start=True, stop=True)
                rinv = work.tile([1, 512], F32, tag="rinv", name="rinv")
                nc.vector.reciprocal_approx_fast(rinv[:], rps[:])
                rinv_r = work.tile([1, 512], F32R, tag="rinv_r", name="rinv_r")
                nc.vector.tensor_copy(rinv_r[:], rinv[:])
                pvs = []
                for cb in range(2):
                    p_cp = work.tile([128, 512], F32, tag=f"pvs{cb}",
                                     name=f"pvs{cb}")
                    nc.vector.tensor_copy(p_cp[:], pvls[t][cb][:])
                    pvs.append(p_cp)
                epi[t] = (rinv_r, pvs)

            def epilogue_b(t):
                rinv_r, pvs = epi.pop(t)
                rbc = ps_small.tile([128, 512], F32, tag="small", name="rbc")
                nc.tensor.matmul(rbc[:], onesrow_r[:], rinv_r[:],
                                 start=True, stop=True)
                for cb in range(2):
                    o_tmp = work.tile([128, 512], F32, tag="o_tmp",
                                      name="o_tmp")
                    nc.vector.tensor_mul(o_tmp[:], pvs[cb][:], rbc[:])
                    o_out = work.tile([128, 512], F32, tag="o_out",
                                      name="o_out")
                    nc.vector.tensor_add(
                        o_out[:], o_tmp[:],
                        xres_sb[:, cb * NQ + t * 512: cb * NQ + (t + 1) * 512])
                    nc.sync.dma_start(
                        out_ext[cb * 128:(cb + 1) * 128,
                                t * 512:(t + 1) * 512],
                        o_out[:])

            for i in range(len(pairs) + 2):
                if i < len(pairs):
                    scores_exp_r(*pairs[i])
                if 1 <= i <= len(pairs):
                    tp, gp = pairs[i - 1]
                    pv_mm(tp, gp)
                    if gp == NG - 1:
                        epilogue_a(tp)
                        if tp < 3:
                            emit_r_tts(tp + 1, 0)
                if 2 <= i <= len(pairs) + 1:
                    tq, gq = pairs[i - 2]
                    if gq == NG - 1:
                        epilogue_b(tq)
    nc.compile()
    return nc


def _get_nc():
    if "nc" not in _cache:
        _cache["nc"] = _build()
    return _cache["nc"]


def _in_maps(x, wq, bq, wk, bk, wv, bv):
    wqt = np.ascontiguousarray(wq.T).astype(ml_dtypes.bfloat16)
    wkt = np.ascontiguousarray(wk.T).astype(ml_dtypes.bfloat16)
    wvt = np.ascontiguousarray(wv.T).astype(ml_dtypes.bfloat16)
    bq4 = np.ascontiguousarray(
        np.tile(np.asarray(bq, np.float32).reshape(D, 1), (4, 1)))
    bk4 = np.ascontiguousarray(
        np.tile(np.asarray(bk, np.float32).reshape(D, 1), (4, 1)))
    maps = []
    for core in range(NCORE):
        b, h = core // 2, core % 2
        xb = np.asarray(x[b], dtype=np.float32).reshape(C, N)
        if h == 1:
            xc = np.concatenate([xb[:, NQ:], xb[:, :NQ]], axis=1)
        else:
            xc = xb
        maps.append({
            "xb": np.ascontiguousarray(xc).astype(ml_dtypes.bfloat16),
            "xres": np.ascontiguousarray(
                xc[:, :NQ] + np.asarray(bv, np.float32).reshape(C, 1)),
            "wqt": wqt, "wkt": wkt, "wvt": wvt,
            "bq4": bq4, "bk4": bk4,
        })
    return maps


def _get_runner():
    """Build the SPMD graph once and cache a reusable jitted executable
    (run_bass_kernel_spmd re-jits per call, paying a full XLA compile)."""
    if "runner" in _cache:
        return _cache["runner"]
    import jax
    from jax.sharding import Mesh, PartitionSpec
    from jax.experimental.shard_map import shard_map
    from concourse import bass2jax, mybir as mb

    nc = _get_nc()
    bass2jax.install_neuronx_cc_hook()
    partition_name = (nc.partition_id_tensor.name
                      if nc.partition_id_tensor else None)
    in_names, out_names, out_avals, zero_shapes = [], [], [], []
    for alloc in nc.m.functions[0].allocations:
        if not isinstance(alloc, mb.MemoryLocationSet):
            continue
        name = alloc.memorylocations[0].name
        if alloc.kind == "ExternalInput":
            if name != partition_name:
                in_names.append(name)
        elif alloc.kind == "ExternalOutput":
            out_names.append(name)
            shape = tuple(alloc.tensor_shape)
            dtype = mb.dt.np(alloc.dtype)
            out_avals.append(jax.core.ShapedArray(shape, dtype))
            zero_shapes.append((shape, dtype))
    n_params = len(in_names)
    full_in_names = list(in_names) + list(out_names)
    if partition_name is not None:
        full_in_names.append(partition_name)
    donate = tuple(range(n_params, n_params + len(out_names)))

    def _body(*args):
        operands = list(args)
        if partition_name is not None:
            operands.append(bass2jax.partition_id_tensor())
        outs = bass2jax._bass_exec_p.bind(
            *operands,
            out_avals=tuple(out_avals),
            in_names=tuple(full_in_names),
            out_names=tuple(out_names),
            lowering_input_output_aliases=(),
            sim_require_finite=True,
            sim_require_nnan=True,
            nc=nc,
        )
        return tuple(outs)

    devices = jax.devices()[:NCORE]
    mesh = Mesh(np.asarray(devices), ("core",))
    in_specs = (PartitionSpec("core"),) * (n_params + len(out_names))
    out_specs = (PartitionSpec("core"),) * len(out_names)
    sharded = jax.jit(
        shard_map(_body, mesh=mesh, in_specs=in_specs, out_specs=out_specs,
                  check_rep=False),
        donate_argnums=donate, keep_unused=True)
    runner = (sharded, in_names, out_names, out_avals, zero_shapes)
    _cache["runner"] = runner
    return runner


def _run_fast(maps):
    sharded, in_names, out_names, out_avals, zero_shapes = _get_runner()
    concat_in = [
        np.concatenate([np.asarray(maps[c][name]) for c in range(NCORE)], axis=0)
        for name in in_names
    ]
    concat_zeros = [
        np.zeros((NCORE * s[0], *s[1:]), dt) for s, dt in zero_shapes
    ]
    out_arrs = sharded(*concat_in, *concat_zeros)
    return [
        {name: np.asarray(out_arrs[i]).reshape(NCORE, *out_avals[i].shape)[c]
         for i, name in enumerate(out_names)}
        for c in range(NCORE)
    ]


def _assemble(results):
    out = np.empty((4, C, N), dtype=np.float32)
    for core in range(NCORE):
        b, h = core // 2, core % 2
        out[b][:, h * NQ:(h + 1) * NQ] = results[core]["out"]
    return out.reshape(4, C, 64, 64)


def _run(inputs, trace=False, tmpdir=None):
    maps = _in_maps(**inputs)
    if trace:
        nc = _get_nc()
        res = run_bass_kernel_spmd(nc, maps, core_ids=list(range(NCORE)),
                                   trace=trace, tmpdir=tmpdir)
        return _assemble(res.results), res
    return _assemble(_run_fast(maps)), None


def kernel(**inputs):
    out, _ = _run(inputs)
    return out
